# revision 1
# baseline (speedup 1.0000x reference)
"""Trainium2 Bass kernel for nn_EngramModule_7378753815202.

kernel(**inputs) takes the FULL (unsharded) inputs as produced by
setup_inputs() and returns the FULL (B, T, D) output.

Strategy: data-parallel over the batch dim — each of the 8 NeuronCores
processes one batch row; the (H, hash_range, E) memory table and the
small MLP weights are replicated to every core. No collectives needed;
per-core outputs are concatenated on the host.

Per-core program (t-tile layout: tile a in [0,32), partition p in
[0,128) -> t = a*128 + p):
  1. n-gram hash indices computed in fp32 exactly like the reference
     (hash_range = 2^18, so the mod is a bitwise AND)
  2. 256 indirect-DMA gathers (128 rows x 256B each) from the table
  3. reduce the 8 (head, n) combos -> seq_sum; PE-transpose; project
     with W_hid^T/H (+ b_hid via a K=1 matmul)
  4. g = hid + mp; z^T = gelu(W_g1 g^T + b_g1) with the bias folded into
     the activation; gate = sigmoid(W_g2 z + b_g2)
  5. out = hid + gate * mp (single fused scalar_tensor_tensor op)
The per-tile stages are software-pipelined (stage2 lags stage1 by one
tile, pair tails by one pair) so the serial SWDGE gather stream on the
Pool engine stays dense.
"""

import numpy as np

B, T, H, E, HR, D, DH = 8, 4096, 4, 64, 262144, 512, 256
NT = T // 128
N_CORES = 8

_CACHE = {}


def _build_nc():
    import concourse.bacc as bacc
    import concourse.mybir as mybir
    import concourse.tile as tile
    from concourse.bass import IndirectOffsetOnAxis

    f32 = mybir.dt.float32
    i32 = mybir.dt.int32
    AF = mybir.ActivationFunctionType
    OP = mybir.AluOpType

    gather_bufs, stag, tail_delay = 6, 1, 1

    nc = bacc.Bacc(
        "TRN2", target_bir_lowering=False, debug=False, num_devices=N_CORES
    )
    tok = nc.dram_tensor("tok", [1, T], i32, kind="ExternalInput")
    hid = nc.dram_tensor("hid", [T, D], f32, kind="ExternalInput")
    emb = nc.dram_tensor("emb", [H * HR, E], f32, kind="ExternalInput")
    w_hid = nc.dram_tensor("w_hid", [D, E], f32, kind="ExternalInput")
    b_hid = nc.dram_tensor("b_hid", [1, D], f32, kind="ExternalInput")
    w_g1 = nc.dram_tensor("w_g1", [DH, D], f32, kind="ExternalInput")
    b_g1 = nc.dram_tensor("b_g1", [1, DH], f32, kind="ExternalInput")
    w_g2 = nc.dram_tensor("w_g2", [1, DH], f32, kind="ExternalInput")
    b_g2 = nc.dram_tensor("b_g2", [1, 1], f32, kind="ExternalInput")
    seeds = nc.dram_tensor("seeds", [1, H], i32, kind="ExternalInput")
    ident_in = nc.dram_tensor("ident", [128, 128], f32, kind="ExternalInput")
    out = nc.dram_tensor("out", [T, D], f32, kind="ExternalOutput")
    tok_pad = nc.dram_tensor("tok_pad", [1, T + 128], i32)

    with tile.TileContext(nc) as tc:
        with (
            tc.tile_pool(name="const", bufs=1) as cp,
            tc.tile_pool(name="psA", bufs=2, space="PSUM") as ppA,
            tc.tile_pool(name="psMP", bufs=1, space="PSUM") as ppMP,
            tc.tile_pool(name="psZ", bufs=2, space="PSUM") as ppZ,
            tc.tile_pool(name="psS", bufs=1, space="PSUM") as ppS,
            tc.tile_pool(name="psG", bufs=2, space="PSUM") as ppG,
            tc.tile_pool(name="work", bufs=4) as wp,
            tc.tile_pool(name="hold", bufs=7) as hp,
            tc.tile_pool(name="gather", bufs=gather_bufs) as gp,
        ):
            ident = cp.tile([128, 128], f32)
            nc.sync.dma_start(out=ident[:], in_=ident_in[:])

            # padded tokens in DRAM so shifted loads stay in bounds
            zpad = cp.tile([1, 128], i32)
            nc.vector.memset(zpad[:], 0)
            nc.sync.dma_start(out=tok_pad[0:1, 0:T], in_=tok[:])
            nc.sync.dma_start(out=tok_pad[0:1, T : T + 128], in_=zpad[:])

            # T0/T1/T2: tok[t+k] as fp32 in (128 p, NT a) layout
            Ts = []
            for k in range(3):
                stg_i = cp.tile([32, 128], i32, tag=f"stgi{k}")
                nc.sync.dma_start(
                    out=stg_i[:],
                    in_=tok_pad[0, k : k + T].rearrange("(a p) -> a p", p=128),
                )
                stg_f = cp.tile([32, 128], f32, tag=f"stgf{k}")
                nc.vector.tensor_copy(out=stg_f[:], in_=stg_i[:])
                ps = ppA.tile([128, 32], f32, tag="tp")
                nc.tensor.transpose(
                    out=ps[:], in_=stg_f[:], identity=ident[0:32, 0:32]
                )
                Tk = cp.tile([128, NT], f32, tag=f"T{k}")
                nc.vector.tensor_copy(out=Tk[:], in_=ps[:])
                Ts.append(Tk)

            # per-head multipliers c_h = float(seed_h + 1), all partitions
            seeds_sb = cp.tile([128, H], i32)
            nc.sync.dma_start(
                out=seeds_sb[:], in_=seeds[:].to_broadcast((128, H))
            )
            seeds_p1 = cp.tile([128, H], i32)
            nc.vector.tensor_scalar_add(seeds_p1[:], seeds_sb[:], 1)
            c_f = cp.tile([128, H], f32)
            nc.vector.tensor_copy(out=c_f[:], in_=seeds_p1[:])

            # hash indices: big_idx[p, a*8 + j], j = h*2 + (n-2)
            big_idx = cp.tile([128, NT * 8], i32)
            bi_view = big_idx[:].rearrange("p (a j) -> p a j", j=8)
            for h in range(H):
                ch = c_f[:, h : h + 1]
                s0 = wp.tile([128, NT], f32, tag="s0")
                s1 = wp.tile([128, NT], f32, tag="s1")
                s2 = wp.tile([128, NT], f32, tag="s2")
                nc.vector.tensor_scalar_mul(s0[:], Ts[0][:], ch)
                nc.vector.tensor_scalar_mul(s1[:], Ts[1][:], ch)
                nc.vector.tensor_scalar_mul(s2[:], Ts[2][:], ch)
                w2 = wp.tile([128, NT], f32, tag="w2")
                nc.vector.tensor_add(w2[:], s0[:], s1[:])
                w3 = wp.tile([128, NT], f32, tag="w3")
                nc.vector.tensor_add(w3[:], w2[:], s2[:])
                for bn, w in ((0, w2), (1, w3)):
                    j = h * 2 + bn
                    wi = wp.tile([128, NT], i32, tag="wi")
                    nc.vector.tensor_copy(out=wi[:], in_=w[:])
                    nc.vector.tensor_scalar(
                        out=bi_view[:, :, j],
                        in0=wi[:],
                        scalar1=HR - 1,
                        scalar2=None,
                        op0=OP.bitwise_and,
                    )

            # W_hid^T / H as (64 e, 512 d)
            wh_stg = cp.tile([128, 4 * E], f32)
            whv = w_hid[:].rearrange("(k p) e -> k p e", p=128)
            for k in range(4):
                nc.sync.dma_start(
                    out=wh_stg[:, k * E : (k + 1) * E], in_=whv[k]
                )
            whT = cp.tile([64, D], f32)
            for k in range(4):
                ps = ppA.tile([64, 128], f32, tag="tp")
                nc.tensor.transpose(
                    out=ps[:],
                    in_=wh_stg[:, k * E : (k + 1) * E],
                    identity=ident[:],
                )
                nc.vector.tensor_scalar_mul(
                    whT[:, k * 128 : (k + 1) * 128], ps[:], 1.0 / H
                )

            # W_g1^T as 4 k-tiles (128 d, 256 h2), stored (128, 4*256)
            wg1_stg = cp.tile([128, 2 * D], f32)
            wg1v = w_g1[:].rearrange("(m p) d -> m p d", p=128)
            for m in range(2):
                nc.sync.dma_start(
                    out=wg1_stg[:, m * D : (m + 1) * D], in_=wg1v[m]
                )
            wg1T = cp.tile([128, 4 * DH], f32)
            for k in range(4):
                for m in range(2):
                    ps = ppA.tile([128, 128], f32, tag="tp")
                    nc.tensor.transpose(
                        out=ps[:],
                        in_=wg1_stg[:, m * D + k * 128 : m * D + (k + 1) * 128],
                        identity=ident[:],
                    )
                    nc.vector.tensor_copy(
                        out=wg1T[:, k * DH + m * 128 : k * DH + (m + 1) * 128],
                        in_=ps[:],
                    )

            # W_g2^T and b_g1^T as (128, 2) column pairs
            wg2_stg = cp.tile([1, DH], f32)
            nc.sync.dma_start(out=wg2_stg[:], in_=w_g2[:])
            bg1_stg = cp.tile([1, DH], f32)
            nc.sync.dma_start(out=bg1_stg[:], in_=b_g1[:])
            wg2T = cp.tile([128, 2], f32)
            bg1T = cp.tile([128, 2], f32)
            for m in range(2):
                ps = ppA.tile([128, 1], f32, tag="tp")
                nc.tensor.transpose(
                    out=ps[:],
                    in_=wg2_stg[0:1, m * 128 : (m + 1) * 128],
                    identity=ident[0:1, 0:1],
                )
                nc.vector.tensor_copy(out=wg2T[:, m : m + 1], in_=ps[:])
                ps2 = ppA.tile([128, 1], f32, tag="tp")
                nc.tensor.transpose(
                    out=ps2[:],
                    in_=bg1_stg[0:1, m * 128 : (m + 1) * 128],
                    identity=ident[0:1, 0:1],
                )
                nc.vector.tensor_copy(out=bg1T[:, m : m + 1], in_=ps2[:])

            # b_hid as a row (added via K=1 matmul); b_g2 broadcast
            bhid_row = cp.tile([1, D], f32)
            nc.sync.dma_start(out=bhid_row[:], in_=b_hid[:])
            ones_row = cp.tile([1, 128], f32)
            nc.vector.memset(ones_row[:], 1.0)
            bg2_bc = cp.tile([128, 1], f32)
            nc.sync.dma_start(
                out=bg2_bc[:], in_=b_g2[:].to_broadcast((128, 1))
            )

            # masks for the final t-tile (invalid n-gram windows)
            mask2 = cp.tile([128, 1], f32)
            nc.vector.tensor_scalar(
                out=mask2[:], in0=ident[:, 127:128], scalar1=-1.0,
                scalar2=1.0, op0=OP.mult, op1=OP.add,
            )
            m3tmp = cp.tile([128, 1], f32)
            nc.vector.tensor_add(
                m3tmp[:], ident[:, 126:127], ident[:, 127:128]
            )
            mask3 = cp.tile([128, 1], f32)
            nc.vector.tensor_scalar(
                out=mask3[:], in0=m3tmp[:], scalar1=-1.0,
                scalar2=1.0, op0=OP.mult, op1=OP.add,
            )

            hidv = hid[:].rearrange("(a p) d -> a p d", p=128)
            outv = out[:].rearrange("(a p) d -> a p d", p=128)

            pair_state = {}

            def emit_tail(st):
                ap_j, ps_zt, mp_sbs, hid_sbs = st
                zg = wp.tile([128, 2 * DH], f32, tag="zg", name="zg")
                for m in range(2):
                    nc.scalar.activation(
                        out=zg[:, m * 2 * 128 : (m + 1) * 2 * 128],
                        in_=ps_zt[:, m * 256 : (m + 1) * 256],
                        func=AF.Gelu,
                        bias=bg1T[:, m : m + 1],
                    )
                ps_s = ppS.tile([128, 2], f32, tag="s", name="ps_s")
                for aoff in range(2):
                    for m in range(2):
                        nc.tensor.matmul(
                            ps_s[:, aoff : aoff + 1],
                            lhsT=zg[
                                :,
                                m * 2 * 128
                                + aoff * 128 : m * 2 * 128
                                + (aoff + 1) * 128,
                            ],
                            rhs=wg2T[:, m : m + 1],
                            start=(m == 0),
                            stop=(m == 1),
                        )
                gate = wp.tile([128, 2], f32, tag="gate", name="gate")
                nc.scalar.activation(
                    out=gate[:], in_=ps_s[:], func=AF.Sigmoid, bias=bg2_bc[:]
                )
                for aoff in range(2):
                    a = 2 * ap_j + aoff
                    o = wp.tile([128, D], f32, tag="o", name="o")
                    nc.vector.scalar_tensor_tensor(
                        out=o[:],
                        in0=mp_sbs[aoff][:],
                        scalar=gate[:, aoff : aoff + 1],
                        in1=hid_sbs[aoff][:],
                        op0=OP.mult,
                        op1=OP.add,
                    )
                    nc.sync.dma_start(out=outv[a], in_=o[:])

            def stage1(a):
                p = a // 2
                st = pair_state.setdefault(
                    p, {"mp": [None, None], "hid": [None, None],
                        "g": [None, None]}
                )
                gbuf = gp.tile([128, 8 * E], f32, tag="gbuf", name="gbuf")
                for j in range(8):
                    h = j // 2
                    nc.gpsimd.indirect_dma_start(
                        out=gbuf[:, j * E : (j + 1) * E],
                        out_offset=None,
                        in_=emb[:],
                        in_offset=IndirectOffsetOnAxis(
                            ap=big_idx[:, a * 8 + j : a * 8 + j + 1], axis=0
                        ),
                        element_offset=h * HR * E,
                    )
                if a == NT - 1:
                    for j in range(8):
                        msk = mask2 if j % 2 == 0 else mask3
                        nc.vector.tensor_scalar_mul(
                            gbuf[:, j * E : (j + 1) * E],
                            gbuf[:, j * E : (j + 1) * E],
                            msk[:],
                        )
                red1 = wp.tile([128, 4 * E], f32, tag="red1", name="red1")
                nc.vector.tensor_add(
                    red1[:], gbuf[:, 0 : 4 * E], gbuf[:, 4 * E : 8 * E]
                )
                red2 = wp.tile([128, 2 * E], f32, tag="red2", name="red2")
                nc.vector.tensor_add(
                    red2[:], red1[:, 0 : 2 * E], red1[:, 2 * E : 4 * E]
                )
                seqs = wp.tile([128, E], f32, tag="seqs", name="seqs")
                nc.vector.tensor_add(
                    seqs[:], red2[:, 0:E], red2[:, E : 2 * E]
                )
                ps_sqT = ppA.tile([64, 128], f32, tag="tp", name="ps_sqT")
                nc.tensor.transpose(
                    out=ps_sqT[:], in_=seqs[:], identity=ident[:]
                )
                sqT = wp.tile([64, 128], f32, tag="sqTs", name="sqT")
                nc.vector.tensor_copy(out=sqT[:], in_=ps_sqT[:])
                ps_mp = ppMP.tile([128, D], f32, tag="mp", name="ps_mp")
                nc.tensor.matmul(
                    ps_mp[:], lhsT=sqT[:], rhs=whT[:], start=True, stop=False
                )
                nc.tensor.matmul(
                    ps_mp[:], lhsT=ones_row[:], rhs=bhid_row[:],
                    start=False, stop=True,
                )
                mp_sb = hp.tile([128, D], f32, tag="mp_s", name="mp_sb")
                nc.vector.tensor_copy(out=mp_sb[:], in_=ps_mp[:])
                st["mp"][a % 2] = mp_sb
                hid_t = hp.tile([128, D], f32, tag="hid", name="hid_t")
                nc.sync.dma_start(out=hid_t[:], in_=hidv[a])
                st["hid"][a % 2] = hid_t
                g = hp.tile([128, D], f32, tag="g", name="g")
                nc.vector.tensor_add(g[:], hid_t[:], mp_sb[:])
                st["g"][a % 2] = g

            def stage2(a):
                p = a // 2
                st = pair_state[p]
                if "zall" not in st:
                    st["zall"] = ppZ.tile(
                        [128, 512], f32, tag="zm", name="ps_zall"
                    )
                ps_zall = st["zall"]
                g = st["g"][a % 2]
                gT = wp.tile([128, D], f32, tag="gT", name="gT")
                ps_g4 = ppG.tile([128, D], f32, tag="g4", name="ps_g4")
                for k in range(4):
                    nc.tensor.transpose(
                        out=ps_g4[:, k * 128 : (k + 1) * 128],
                        in_=g[:, k * 128 : (k + 1) * 128],
                        identity=ident[:],
                    )
                nc.vector.tensor_copy(out=gT[:], in_=ps_g4[:])
                aoff = a % 2
                for m in range(2):
                    for k in range(4):
                        nc.tensor.matmul(
                            ps_zall[
                                :,
                                m * 256 + aoff * 128 : m * 256 + (aoff + 1) * 128,
                            ],
                            lhsT=wg1T[
                                :, k * DH + m * 128 : k * DH + (m + 1) * 128
                            ],
                            rhs=gT[:, k * 128 : (k + 1) * 128],
                            start=(k == 0),
                            stop=(k == 3),
                        )

            def tail(p):
                st = pair_state.pop(p)
                emit_tail((p, st["zall"], st["mp"], st["hid"]))

            for a in range(NT + stag):
                if a < NT:
                    stage1(a)
                a2 = a - stag
                if 0 <= a2 < NT:
                    stage2(a2)
                    if a2 % 2 == 1:
                        pdone = a2 // 2
                        if pdone - tail_delay >= 0:
                            tail(pdone - tail_delay)
            for p in range(NT // 2 - tail_delay, NT // 2):
                tail(p)

    nc.compile()
    return nc


class _Runner:
    """PJRT runner (axon) for the prebuilt Bass module: emb + weights
    replicated to all cores, tok/hid sharded along the batch axis."""

    REPLICATED = {"emb", "w_hid", "b_hid", "w_g1", "b_g1", "w_g2", "b_g2",
                  "seeds", "ident"}

    def __init__(self, nc):
        import jax
        from jax.sharding import Mesh, NamedSharding, PartitionSpec
        from jax.experimental.shard_map import shard_map
        import concourse.mybir as mybir
        from concourse import bass2jax

        self.jax = jax
        self.NamedSharding = NamedSharding
        self.PartitionSpec = PartitionSpec
        bass2jax.install_neuronx_cc_hook()
        self.nc = nc
        partition_name = (
            nc.partition_id_tensor.name if nc.partition_id_tensor else None
        )
        in_names, out_names, out_avals, zero_outs = [], [], [], []
        for alloc in nc.m.functions[0].allocations:
            if not isinstance(alloc, mybir.MemoryLocationSet):
                continue
            name = alloc.memorylocations[0].name
            if alloc.kind == "ExternalInput":
                if name != partition_name:
                    in_names.append(name)
            elif alloc.kind == "ExternalOutput":
                out_names.append(name)
                shape = tuple(alloc.tensor_shape)
                dtype = mybir.dt.np(alloc.dtype)
                out_avals.append(jax.core.ShapedArray(shape, dtype))
                zero_outs.append(np.zeros(shape, dtype))
        self.in_names = in_names
        self.out_names = out_names
        self.out_avals = out_avals
        self.zero_outs = zero_outs
        n_params = len(in_names)
        n_outs = len(out_avals)
        all_names = list(in_names) + list(out_names)
        if partition_name is not None:
            all_names.append(partition_name)
        all_names = tuple(all_names)

        def _body(*args):
            operands = list(args)
            if partition_name is not None:
                operands.append(bass2jax.partition_id_tensor())
            outs = bass2jax._bass_exec_p.bind(
                *operands,
                out_avals=tuple(out_avals),
                in_names=all_names,
                out_names=tuple(out_names),
                lowering_input_output_aliases=(),
                sim_require_finite=True,
                sim_require_nnan=True,
                nc=nc,
            )
            return tuple(outs)

        devices = jax.devices()[:N_CORES]
        self.mesh = Mesh(np.asarray(devices), ("core",))
        in_specs = tuple(
            PartitionSpec() if name in self.REPLICATED
            else PartitionSpec("core")
            for name in in_names
        ) + (PartitionSpec("core"),) * n_outs
        out_specs = (PartitionSpec("core"),) * n_outs
        self.fn = jax.jit(
            shard_map(
                _body, mesh=self.mesh, in_specs=in_specs,
                out_specs=out_specs, check_rep=False,
            ),
            donate_argnums=tuple(range(n_params, n_params + n_outs)),
            keep_unused=True,
        )

    def _sharding(self, name=None):
        if name is not None and name in self.REPLICATED:
            return self.NamedSharding(self.mesh, self.PartitionSpec())
        return self.NamedSharding(self.mesh, self.PartitionSpec("core"))

    def put_inputs(self, per_core, replicated_map):
        arrs = []
        for name in self.in_names:
            if name in self.REPLICATED:
                a = replicated_map[name]
            else:
                a = np.concatenate([m[name] for m in per_core], axis=0)
            arrs.append(self.jax.device_put(a, self._sharding(name)))
        self.jax.block_until_ready(arrs)
        return arrs

    def put_zeros(self):
        zs = []
        for z in self.zero_outs:
            full = np.zeros((N_CORES * z.shape[0], *z.shape[1:]), z.dtype)
            zs.append(self.jax.device_put(full, self._sharding()))
        self.jax.block_until_ready(zs)
        return zs

    def run(self, dev_inputs):
        outs = self.fn(*dev_inputs, *self.put_zeros())
        self.jax.block_until_ready(outs)
        full = np.asarray(outs[0]).reshape(N_CORES, T, D)
        return full


def _get_runner():
    if "runner" not in _CACHE:
        nc = _build_nc()
        _CACHE["runner"] = _Runner(nc)
    return _CACHE["runner"]


def kernel(token_ids, hidden_state, embeddings, W_hid, b_hid, W_g1, b_g1,
           W_g2, b_g2, seeds, hash_range, max_n):
    token_ids = np.asarray(token_ids, np.int32)
    hidden_state = np.asarray(hidden_state, np.float32)
    embeddings = np.asarray(embeddings, np.float32)
    assert int(hash_range) == HR and int(max_n) == 3
    assert token_ids.shape == (B, T) and hidden_state.shape == (B, T, D)

    replicated = {
        "emb": embeddings.reshape(H * HR, E),
        "w_hid": np.asarray(W_hid, np.float32).reshape(D, E),
        "b_hid": np.asarray(b_hid, np.float32).reshape(1, D),
        "w_g1": np.asarray(W_g1, np.float32).reshape(DH, D),
        "b_g1": np.asarray(b_g1, np.float32).reshape(1, DH),
        "w_g2": np.asarray(W_g2, np.float32).reshape(1, DH),
        "b_g2": np.asarray(b_g2, np.float32).reshape(1, 1),
        "seeds": np.asarray(seeds, np.int32).reshape(1, H),
        "ident": np.eye(128, dtype=np.float32),
    }
    per_core = [
        {"tok": token_ids[c : c + 1], "hid": hidden_state[c]}
        for c in range(N_CORES)
    ]

    r = _get_runner()
    dev = r.put_inputs(per_core, replicated)
    return r.run(dev)


# revision 2
# speedup vs baseline: 1.0248x; 1.0248x over previous
"""Trainium2 Bass kernel for nn_EngramModule_7378753815202.

kernel(**inputs) takes the FULL (unsharded) inputs as produced by
setup_inputs() and returns the FULL (B, T, D) output.

Strategy: data-parallel over the batch dim — each of the 8 NeuronCores
processes one batch row; the (H, hash_range, E) memory table and the
small MLP weights are replicated to every core. No collectives needed;
per-core outputs are concatenated on the host.

Per-core program (t-tile layout: tile a in [0,32), partition p in
[0,128) -> t = a*128 + p):
  1. n-gram hash indices computed in fp32 exactly like the reference
     (hash_range = 2^18, so the mod is a bitwise AND)
  2. 256 indirect-DMA gathers (128 rows x 256B each) from the table
  3. reduce the 8 (head, n) combos -> seq_sum; PE-transpose; project
     with W_hid^T/H (+ b_hid via a K=1 matmul)
  4. g = hid + mp; z^T = gelu(W_g1 g^T + b_g1) with the bias folded into
     the activation; gate = sigmoid(W_g2 z + b_g2)
  5. out = hid + gate * mp (single fused scalar_tensor_tensor op)
The per-tile stages are software-pipelined (stage2 lags stage1 by one
tile, pair tails by one pair) so the serial SWDGE gather stream on the
Pool engine stays dense.
"""

import numpy as np

B, T, H, E, HR, D, DH = 8, 4096, 4, 64, 262144, 512, 256
NT = T // 128
N_CORES = 8

_CACHE = {}


def _build_nc():
    import concourse.bacc as bacc
    import concourse.mybir as mybir
    import concourse.tile as tile
    from concourse.bass import IndirectOffsetOnAxis

    f32 = mybir.dt.float32
    i32 = mybir.dt.int32
    AF = mybir.ActivationFunctionType
    OP = mybir.AluOpType

    gather_bufs, stag, tail_delay = 10, 2, 1

    nc = bacc.Bacc(
        "TRN2", target_bir_lowering=False, debug=False, num_devices=N_CORES
    )
    tok = nc.dram_tensor("tok", [1, T], i32, kind="ExternalInput")
    hid = nc.dram_tensor("hid", [T, D], f32, kind="ExternalInput")
    emb = nc.dram_tensor("emb", [H * HR, E], f32, kind="ExternalInput")
    w_hid = nc.dram_tensor("w_hid", [D, E], f32, kind="ExternalInput")
    b_hid = nc.dram_tensor("b_hid", [1, D], f32, kind="ExternalInput")
    w_g1 = nc.dram_tensor("w_g1", [DH, D], f32, kind="ExternalInput")
    b_g1 = nc.dram_tensor("b_g1", [1, DH], f32, kind="ExternalInput")
    w_g2 = nc.dram_tensor("w_g2", [1, DH], f32, kind="ExternalInput")
    b_g2 = nc.dram_tensor("b_g2", [1, 1], f32, kind="ExternalInput")
    seeds = nc.dram_tensor("seeds", [1, H], i32, kind="ExternalInput")
    ident_in = nc.dram_tensor("ident", [128, 128], f32, kind="ExternalInput")
    out = nc.dram_tensor("out", [T, D], f32, kind="ExternalOutput")
    tok_pad = nc.dram_tensor("tok_pad", [1, T + 128], i32)

    with tile.TileContext(nc) as tc:
        with (
            tc.tile_pool(name="const", bufs=1) as cp,
            tc.tile_pool(name="psA", bufs=1, space="PSUM") as ppA,
            tc.tile_pool(name="psMP", bufs=1, space="PSUM") as ppMP,
            tc.tile_pool(name="psZ", bufs=3, space="PSUM") as ppZ,
            tc.tile_pool(name="psS", bufs=1, space="PSUM") as ppS,
            tc.tile_pool(name="psG", bufs=2, space="PSUM") as ppG,
            tc.tile_pool(name="work", bufs=5) as wp,
            tc.tile_pool(name="hold", bufs=9) as hp,
            tc.tile_pool(name="gather", bufs=gather_bufs) as gp,
        ):
            ident = cp.tile([128, 128], f32)
            nc.sync.dma_start(out=ident[:], in_=ident_in[:])

            # padded tokens in DRAM so shifted loads stay in bounds
            zpad = cp.tile([1, 128], i32)
            nc.vector.memset(zpad[:], 0)
            nc.sync.dma_start(out=tok_pad[0:1, 0:T], in_=tok[:])
            nc.sync.dma_start(out=tok_pad[0:1, T : T + 128], in_=zpad[:])

            # T0/T1/T2: tok[t+k] as fp32 in (128 p, NT a) layout
            Ts = []
            for k in range(3):
                stg_i = cp.tile([32, 128], i32, tag=f"stgi{k}")
                nc.sync.dma_start(
                    out=stg_i[:],
                    in_=tok_pad[0, k : k + T].rearrange("(a p) -> a p", p=128),
                )
                stg_f = cp.tile([32, 128], f32, tag=f"stgf{k}")
                nc.vector.tensor_copy(out=stg_f[:], in_=stg_i[:])
                ps = ppA.tile([128, 32], f32, tag="tp")
                nc.tensor.transpose(
                    out=ps[:], in_=stg_f[:], identity=ident[0:32, 0:32]
                )
                Tk = cp.tile([128, NT], f32, tag=f"T{k}")
                nc.vector.tensor_copy(out=Tk[:], in_=ps[:])
                Ts.append(Tk)

            # per-head multipliers c_h = float(seed_h + 1), all partitions
            seeds_sb = cp.tile([128, H], i32)
            nc.sync.dma_start(
                out=seeds_sb[:], in_=seeds[:].to_broadcast((128, H))
            )
            seeds_p1 = cp.tile([128, H], i32)
            nc.vector.tensor_scalar_add(seeds_p1[:], seeds_sb[:], 1)
            c_f = cp.tile([128, H], f32)
            nc.vector.tensor_copy(out=c_f[:], in_=seeds_p1[:])

            # hash indices: big_idx[p, a*8 + j], j = h*2 + (n-2)
            big_idx = cp.tile([128, NT * 8], i32)
            bi_view = big_idx[:].rearrange("p (a j) -> p a j", j=8)
            for h in range(H):
                ch = c_f[:, h : h + 1]
                s0 = wp.tile([128, NT], f32, tag="s0")
                s1 = wp.tile([128, NT], f32, tag="s1")
                s2 = wp.tile([128, NT], f32, tag="s2")
                nc.vector.tensor_scalar_mul(s0[:], Ts[0][:], ch)
                nc.vector.tensor_scalar_mul(s1[:], Ts[1][:], ch)
                nc.vector.tensor_scalar_mul(s2[:], Ts[2][:], ch)
                w2 = wp.tile([128, NT], f32, tag="w2")
                nc.vector.tensor_add(w2[:], s0[:], s1[:])
                w3 = wp.tile([128, NT], f32, tag="w3")
                nc.vector.tensor_add(w3[:], w2[:], s2[:])
                for bn, w in ((0, w2), (1, w3)):
                    j = h * 2 + bn
                    wi = wp.tile([128, NT], i32, tag="wi")
                    nc.vector.tensor_copy(out=wi[:], in_=w[:])
                    nc.vector.tensor_scalar(
                        out=bi_view[:, :, j],
                        in0=wi[:],
                        scalar1=HR - 1,
                        scalar2=None,
                        op0=OP.bitwise_and,
                    )

            # W_hid^T / H as (64 e, 512 d)
            wh_stg = cp.tile([128, 4 * E], f32)
            whv = w_hid[:].rearrange("(k p) e -> k p e", p=128)
            for k in range(4):
                nc.sync.dma_start(
                    out=wh_stg[:, k * E : (k + 1) * E], in_=whv[k]
                )
            whT = cp.tile([64, D], f32)
            for k in range(4):
                ps = ppA.tile([64, 128], f32, tag="tp")
                nc.tensor.transpose(
                    out=ps[:],
                    in_=wh_stg[:, k * E : (k + 1) * E],
                    identity=ident[:],
                )
                nc.vector.tensor_scalar_mul(
                    whT[:, k * 128 : (k + 1) * 128], ps[:], 1.0 / H
                )

            # W_g1^T as 4 k-tiles (128 d, 256 h2), stored (128, 4*256)
            wg1_stg = cp.tile([128, 2 * D], f32)
            wg1v = w_g1[:].rearrange("(m p) d -> m p d", p=128)
            for m in range(2):
                nc.sync.dma_start(
                    out=wg1_stg[:, m * D : (m + 1) * D], in_=wg1v[m]
                )
            wg1T = cp.tile([128, 4 * DH], f32)
            for k in range(4):
                for m in range(2):
                    ps = ppA.tile([128, 128], f32, tag="tp")
                    nc.tensor.transpose(
                        out=ps[:],
                        in_=wg1_stg[:, m * D + k * 128 : m * D + (k + 1) * 128],
                        identity=ident[:],
                    )
                    nc.vector.tensor_copy(
                        out=wg1T[:, k * DH + m * 128 : k * DH + (m + 1) * 128],
                        in_=ps[:],
                    )

            # W_g2^T and b_g1^T as (128, 2) column pairs
            wg2_stg = cp.tile([1, DH], f32)
            nc.sync.dma_start(out=wg2_stg[:], in_=w_g2[:])
            bg1_stg = cp.tile([1, DH], f32)
            nc.sync.dma_start(out=bg1_stg[:], in_=b_g1[:])
            wg2T = cp.tile([128, 2], f32)
            bg1T = cp.tile([128, 2], f32)
            for m in range(2):
                ps = ppA.tile([128, 1], f32, tag="tp")
                nc.tensor.transpose(
                    out=ps[:],
                    in_=wg2_stg[0:1, m * 128 : (m + 1) * 128],
                    identity=ident[0:1, 0:1],
                )
                nc.vector.tensor_copy(out=wg2T[:, m : m + 1], in_=ps[:])
                ps2 = ppA.tile([128, 1], f32, tag="tp")
                nc.tensor.transpose(
                    out=ps2[:],
                    in_=bg1_stg[0:1, m * 128 : (m + 1) * 128],
                    identity=ident[0:1, 0:1],
                )
                nc.vector.tensor_copy(out=bg1T[:, m : m + 1], in_=ps2[:])

            # b_hid as a row (added via K=1 matmul); b_g2 broadcast
            bhid_row = cp.tile([1, D], f32)
            nc.sync.dma_start(out=bhid_row[:], in_=b_hid[:])
            ones_row = cp.tile([1, 128], f32)
            nc.vector.memset(ones_row[:], 1.0)
            bg2_bc = cp.tile([128, 1], f32)
            nc.sync.dma_start(
                out=bg2_bc[:], in_=b_g2[:].to_broadcast((128, 1))
            )

            # masks for the final t-tile (invalid n-gram windows)
            mask2 = cp.tile([128, 1], f32)
            nc.vector.tensor_scalar(
                out=mask2[:], in0=ident[:, 127:128], scalar1=-1.0,
                scalar2=1.0, op0=OP.mult, op1=OP.add,
            )
            m3tmp = cp.tile([128, 1], f32)
            nc.vector.tensor_add(
                m3tmp[:], ident[:, 126:127], ident[:, 127:128]
            )
            mask3 = cp.tile([128, 1], f32)
            nc.vector.tensor_scalar(
                out=mask3[:], in0=m3tmp[:], scalar1=-1.0,
                scalar2=1.0, op0=OP.mult, op1=OP.add,
            )

            hidv = hid[:].rearrange("(a p) d -> a p d", p=128)
            outv = out[:].rearrange("(a p) d -> a p d", p=128)

            pair_state = {}

            def emit_tail(st):
                ap_j, ps_zt, mp_sbs, hid_sbs = st
                zg = wp.tile([128, 2 * DH], f32, tag="zg", name="zg")
                for m in range(2):
                    nc.scalar.activation(
                        out=zg[:, m * 2 * 128 : (m + 1) * 2 * 128],
                        in_=ps_zt[:, m * 256 : (m + 1) * 256],
                        func=AF.Gelu,
                        bias=bg1T[:, m : m + 1],
                    )
                ps_s = ppS.tile([128, 2], f32, tag="s", name="ps_s")
                for aoff in range(2):
                    for m in range(2):
                        nc.tensor.matmul(
                            ps_s[:, aoff : aoff + 1],
                            lhsT=zg[
                                :,
                                m * 2 * 128
                                + aoff * 128 : m * 2 * 128
                                + (aoff + 1) * 128,
                            ],
                            rhs=wg2T[:, m : m + 1],
                            start=(m == 0),
                            stop=(m == 1),
                        )
                gate = wp.tile([128, 2], f32, tag="gate", name="gate")
                nc.scalar.activation(
                    out=gate[:], in_=ps_s[:], func=AF.Sigmoid, bias=bg2_bc[:]
                )
                for aoff in range(2):
                    a = 2 * ap_j + aoff
                    o = wp.tile([128, D], f32, tag="o", name="o")
                    nc.vector.scalar_tensor_tensor(
                        out=o[:],
                        in0=mp_sbs[aoff][:],
                        scalar=gate[:, aoff : aoff + 1],
                        in1=hid_sbs[aoff][:],
                        op0=OP.mult,
                        op1=OP.add,
                    )
                    nc.sync.dma_start(out=outv[a], in_=o[:])

            def stage1(a):
                p = a // 2
                st = pair_state.setdefault(
                    p, {"mp": [None, None], "hid": [None, None],
                        "g": [None, None]}
                )
                gbuf = gp.tile([128, 8 * E], f32, tag="gbuf", name="gbuf")
                for j in range(8):
                    h = j // 2
                    nc.gpsimd.indirect_dma_start(
                        out=gbuf[:, j * E : (j + 1) * E],
                        out_offset=None,
                        in_=emb[:],
                        in_offset=IndirectOffsetOnAxis(
                            ap=big_idx[:, a * 8 + j : a * 8 + j + 1], axis=0
                        ),
                        element_offset=h * HR * E,
                    )
                if a == NT - 1:
                    for j in range(8):
                        msk = mask2 if j % 2 == 0 else mask3
                        nc.vector.tensor_scalar_mul(
                            gbuf[:, j * E : (j + 1) * E],
                            gbuf[:, j * E : (j + 1) * E],
                            msk[:],
                        )
                red1 = wp.tile([128, 4 * E], f32, tag="red1", name="red1")
                nc.vector.tensor_add(
                    red1[:], gbuf[:, 0 : 4 * E], gbuf[:, 4 * E : 8 * E]
                )
                red2 = wp.tile([128, 2 * E], f32, tag="red2", name="red2")
                nc.vector.tensor_add(
                    red2[:], red1[:, 0 : 2 * E], red1[:, 2 * E : 4 * E]
                )
                seqs = wp.tile([128, E], f32, tag="seqs", name="seqs")
                nc.vector.tensor_add(
                    seqs[:], red2[:, 0:E], red2[:, E : 2 * E]
                )
                ps_sqT = ppA.tile([64, 128], f32, tag="tp", name="ps_sqT")
                nc.tensor.transpose(
                    out=ps_sqT[:], in_=seqs[:], identity=ident[:]
                )
                sqT = wp.tile([64, 128], f32, tag="sqTs", name="sqT")
                nc.vector.tensor_copy(out=sqT[:], in_=ps_sqT[:])
                ps_mp = ppMP.tile([128, D], f32, tag="mp", name="ps_mp")
                nc.tensor.matmul(
                    ps_mp[:], lhsT=sqT[:], rhs=whT[:], start=True, stop=False
                )
                nc.tensor.matmul(
                    ps_mp[:], lhsT=ones_row[:], rhs=bhid_row[:],
                    start=False, stop=True,
                )
                mp_sb = hp.tile([128, D], f32, tag="mp_s", name="mp_sb")
                nc.vector.tensor_copy(out=mp_sb[:], in_=ps_mp[:])
                st["mp"][a % 2] = mp_sb
                hid_t = hp.tile([128, D], f32, tag="hid", name="hid_t")
                nc.sync.dma_start(out=hid_t[:], in_=hidv[a])
                st["hid"][a % 2] = hid_t
                g = hp.tile([128, D], f32, tag="g", name="g")
                nc.vector.tensor_add(g[:], hid_t[:], mp_sb[:])
                st["g"][a % 2] = g

            def stage2(a):
                p = a // 2
                st = pair_state[p]
                if "zall" not in st:
                    st["zall"] = ppZ.tile(
                        [128, 512], f32, tag="zm", name="ps_zall"
                    )
                ps_zall = st["zall"]
                g = st["g"][a % 2]
                gT = wp.tile([128, D], f32, tag="gT", name="gT")
                ps_g4 = ppG.tile([128, D], f32, tag="g4", name="ps_g4")
                for k in range(4):
                    nc.tensor.transpose(
                        out=ps_g4[:, k * 128 : (k + 1) * 128],
                        in_=g[:, k * 128 : (k + 1) * 128],
                        identity=ident[:],
                    )
                nc.vector.tensor_copy(out=gT[:], in_=ps_g4[:])
                aoff = a % 2
                for m in range(2):
                    for k in range(4):
                        nc.tensor.matmul(
                            ps_zall[
                                :,
                                m * 256 + aoff * 128 : m * 256 + (aoff + 1) * 128,
                            ],
                            lhsT=wg1T[
                                :, k * DH + m * 128 : k * DH + (m + 1) * 128
                            ],
                            rhs=gT[:, k * 128 : (k + 1) * 128],
                            start=(k == 0),
                            stop=(k == 3),
                        )

            def tail(p):
                st = pair_state.pop(p)
                emit_tail((p, st["zall"], st["mp"], st["hid"]))

            for a in range(NT + stag):
                if a < NT:
                    stage1(a)
                a2 = a - stag
                if 0 <= a2 < NT:
                    stage2(a2)
                    if a2 % 2 == 1:
                        pdone = a2 // 2
                        if pdone - tail_delay >= 0:
                            tail(pdone - tail_delay)
            for p in range(NT // 2 - tail_delay, NT // 2):
                tail(p)

    nc.compile()
    return nc


class _Runner:
    """PJRT runner (axon) for the prebuilt Bass module: emb + weights
    replicated to all cores, tok/hid sharded along the batch axis."""

    REPLICATED = {"emb", "w_hid", "b_hid", "w_g1", "b_g1", "w_g2", "b_g2",
                  "seeds", "ident"}

    def __init__(self, nc):
        import jax
        from jax.sharding import Mesh, NamedSharding, PartitionSpec
        from jax.experimental.shard_map import shard_map
        import concourse.mybir as mybir
        from concourse import bass2jax

        self.jax = jax
        self.NamedSharding = NamedSharding
        self.PartitionSpec = PartitionSpec
        bass2jax.install_neuronx_cc_hook()
        self.nc = nc
        partition_name = (
            nc.partition_id_tensor.name if nc.partition_id_tensor else None
        )
        in_names, out_names, out_avals, zero_outs = [], [], [], []
        for alloc in nc.m.functions[0].allocations:
            if not isinstance(alloc, mybir.MemoryLocationSet):
                continue
            name = alloc.memorylocations[0].name
            if alloc.kind == "ExternalInput":
                if name != partition_name:
                    in_names.append(name)
            elif alloc.kind == "ExternalOutput":
                out_names.append(name)
                shape = tuple(alloc.tensor_shape)
                dtype = mybir.dt.np(alloc.dtype)
                out_avals.append(jax.core.ShapedArray(shape, dtype))
                zero_outs.append(np.zeros(shape, dtype))
        self.in_names = in_names
        self.out_names = out_names
        self.out_avals = out_avals
        self.zero_outs = zero_outs
        n_params = len(in_names)
        n_outs = len(out_avals)
        all_names = list(in_names) + list(out_names)
        if partition_name is not None:
            all_names.append(partition_name)
        all_names = tuple(all_names)

        def _body(*args):
            operands = list(args)
            if partition_name is not None:
                operands.append(bass2jax.partition_id_tensor())
            outs = bass2jax._bass_exec_p.bind(
                *operands,
                out_avals=tuple(out_avals),
                in_names=all_names,
                out_names=tuple(out_names),
                lowering_input_output_aliases=(),
                sim_require_finite=True,
                sim_require_nnan=True,
                nc=nc,
            )
            return tuple(outs)

        devices = jax.devices()[:N_CORES]
        self.mesh = Mesh(np.asarray(devices), ("core",))
        in_specs = tuple(
            PartitionSpec() if name in self.REPLICATED
            else PartitionSpec("core")
            for name in in_names
        ) + (PartitionSpec("core"),) * n_outs
        out_specs = (PartitionSpec("core"),) * n_outs
        self.fn = jax.jit(
            shard_map(
                _body, mesh=self.mesh, in_specs=in_specs,
                out_specs=out_specs, check_rep=False,
            ),
            donate_argnums=tuple(range(n_params, n_params + n_outs)),
            keep_unused=True,
        )

    def _sharding(self, name=None):
        if name is not None and name in self.REPLICATED:
            return self.NamedSharding(self.mesh, self.PartitionSpec())
        return self.NamedSharding(self.mesh, self.PartitionSpec("core"))

    def put_inputs(self, per_core, replicated_map):
        arrs = []
        for name in self.in_names:
            if name in self.REPLICATED:
                a = replicated_map[name]
            else:
                a = np.concatenate([m[name] for m in per_core], axis=0)
            arrs.append(self.jax.device_put(a, self._sharding(name)))
        self.jax.block_until_ready(arrs)
        return arrs

    def put_zeros(self):
        zs = []
        for z in self.zero_outs:
            full = np.zeros((N_CORES * z.shape[0], *z.shape[1:]), z.dtype)
            zs.append(self.jax.device_put(full, self._sharding()))
        self.jax.block_until_ready(zs)
        return zs

    def run(self, dev_inputs):
        outs = self.fn(*dev_inputs, *self.put_zeros())
        self.jax.block_until_ready(outs)
        full = np.asarray(outs[0]).reshape(N_CORES, T, D)
        return full


def _get_runner():
    if "runner" not in _CACHE:
        nc = _build_nc()
        _CACHE["runner"] = _Runner(nc)
    return _CACHE["runner"]


def kernel(token_ids, hidden_state, embeddings, W_hid, b_hid, W_g1, b_g1,
           W_g2, b_g2, seeds, hash_range, max_n):
    token_ids = np.asarray(token_ids, np.int32)
    hidden_state = np.asarray(hidden_state, np.float32)
    embeddings = np.asarray(embeddings, np.float32)
    assert int(hash_range) == HR and int(max_n) == 3
    assert token_ids.shape == (B, T) and hidden_state.shape == (B, T, D)

    replicated = {
        "emb": embeddings.reshape(H * HR, E),
        "w_hid": np.asarray(W_hid, np.float32).reshape(D, E),
        "b_hid": np.asarray(b_hid, np.float32).reshape(1, D),
        "w_g1": np.asarray(W_g1, np.float32).reshape(DH, D),
        "b_g1": np.asarray(b_g1, np.float32).reshape(1, DH),
        "w_g2": np.asarray(W_g2, np.float32).reshape(1, DH),
        "b_g2": np.asarray(b_g2, np.float32).reshape(1, 1),
        "seeds": np.asarray(seeds, np.int32).reshape(1, H),
        "ident": np.eye(128, dtype=np.float32),
    }
    per_core = [
        {"tok": token_ids[c : c + 1], "hid": hidden_state[c]}
        for c in range(N_CORES)
    ]

    r = _get_runner()
    dev = r.put_inputs(per_core, replicated)
    return r.run(dev)


# revision 3
# speedup vs baseline: 1.0341x; 1.0091x over previous
"""Trainium2 Bass kernel for nn_EngramModule_7378753815202.

kernel(**inputs) takes the FULL (unsharded) inputs as produced by
setup_inputs() and returns the FULL (B, T, D) output.

Strategy: data-parallel over the batch dim — each of the 8 NeuronCores
processes one batch row; the (H, hash_range, E) memory table and the
small MLP weights are replicated to every core. No collectives needed;
per-core outputs are concatenated on the host.

Per-core program (t-tile layout: tile a in [0,32), partition p in
[0,128) -> t = a*128 + p):
  1. n-gram hash indices computed in fp32 exactly like the reference
     (hash_range = 2^18, so the mod is a bitwise AND)
  2. 256 indirect-DMA gathers (128 rows x 256B each) from the table
  3. reduce the 8 (head, n) combos -> seq_sum; PE-transpose; project
     with W_hid^T/H (+ b_hid via a K=1 matmul)
  4. g = hid + mp; z^T = gelu(W_g1 g^T + b_g1) with the bias folded into
     the activation; gate = sigmoid(W_g2 z + b_g2)
  5. out = hid + gate * mp (single fused scalar_tensor_tensor op)
The per-tile stages are software-pipelined (stage2 lags stage1 by one
tile, pair tails by one pair) so the serial SWDGE gather stream on the
Pool engine stays dense.
"""

import numpy as np

B, T, H, E, HR, D, DH = 8, 4096, 4, 64, 262144, 512, 256
NT = T // 128
N_CORES = 8

_CACHE = {}


def _build_nc():
    import concourse.bacc as bacc
    import concourse.mybir as mybir
    import concourse.tile as tile
    from concourse.bass import IndirectOffsetOnAxis

    f32 = mybir.dt.float32
    i32 = mybir.dt.int32
    AF = mybir.ActivationFunctionType
    OP = mybir.AluOpType

    gather_bufs, stag, tail_delay = 10, 2, 1

    nc = bacc.Bacc(
        "TRN2", target_bir_lowering=False, debug=False, num_devices=N_CORES
    )
    tok = nc.dram_tensor("tok", [1, T], i32, kind="ExternalInput")
    hid = nc.dram_tensor("hid", [T, D], f32, kind="ExternalInput")
    emb = nc.dram_tensor("emb", [H * HR, E], f32, kind="ExternalInput")
    w_hid = nc.dram_tensor("w_hid", [D, E], f32, kind="ExternalInput")
    b_hid = nc.dram_tensor("b_hid", [1, D], f32, kind="ExternalInput")
    w_g1 = nc.dram_tensor("w_g1", [DH, D], f32, kind="ExternalInput")
    b_g1 = nc.dram_tensor("b_g1", [1, DH], f32, kind="ExternalInput")
    w_g2 = nc.dram_tensor("w_g2", [1, DH], f32, kind="ExternalInput")
    b_g2 = nc.dram_tensor("b_g2", [1, 1], f32, kind="ExternalInput")
    seeds = nc.dram_tensor("seeds", [1, H], i32, kind="ExternalInput")
    ident_in = nc.dram_tensor("ident", [128, 128], f32, kind="ExternalInput")
    out = nc.dram_tensor("out", [T, D], f32, kind="ExternalOutput")
    tok_pad = nc.dram_tensor("tok_pad", [1, T + 128], i32)

    with tile.TileContext(nc) as tc:
        with (
            tc.tile_pool(name="const", bufs=1) as cp,
            tc.tile_pool(name="psA", bufs=1, space="PSUM") as ppA,
            tc.tile_pool(name="psMP", bufs=1, space="PSUM") as ppMP,
            tc.tile_pool(name="psZ", bufs=3, space="PSUM") as ppZ,
            tc.tile_pool(name="psS", bufs=1, space="PSUM") as ppS,
            tc.tile_pool(name="psG", bufs=2, space="PSUM") as ppG,
            tc.tile_pool(name="work", bufs=5) as wp,
            tc.tile_pool(name="hold", bufs=9) as hp,
            tc.tile_pool(name="gather", bufs=gather_bufs) as gp,
        ):
            ident = cp.tile([128, 128], f32)
            nc.sync.dma_start(out=ident[:], in_=ident_in[:])

            # padded tokens in DRAM so shifted loads stay in bounds
            zpad = cp.tile([1, 128], i32)
            nc.vector.memset(zpad[:], 0)
            nc.sync.dma_start(out=tok_pad[0:1, 0:T], in_=tok[:])
            nc.sync.dma_start(out=tok_pad[0:1, T : T + 128], in_=zpad[:])

            # T0/T1/T2: tok[t+k] as fp32 in (128 p, NT a) layout
            Ts = []
            for k in range(3):
                stg_i = cp.tile([32, 128], i32, tag=f"stgi{k}")
                nc.sync.dma_start(
                    out=stg_i[:],
                    in_=tok_pad[0, k : k + T].rearrange("(a p) -> a p", p=128),
                )
                stg_f = cp.tile([32, 128], f32, tag=f"stgf{k}")
                nc.vector.tensor_copy(out=stg_f[:], in_=stg_i[:])
                ps = ppA.tile([128, 32], f32, tag="tp")
                nc.tensor.transpose(
                    out=ps[:], in_=stg_f[:], identity=ident[0:32, 0:32]
                )
                Tk = cp.tile([128, NT], f32, tag=f"T{k}")
                nc.vector.tensor_copy(out=Tk[:], in_=ps[:])
                Ts.append(Tk)

            # per-head multipliers c_h = float(seed_h + 1), all partitions
            seeds_sb = cp.tile([128, H], i32)
            nc.sync.dma_start(
                out=seeds_sb[:], in_=seeds[:].to_broadcast((128, H))
            )
            seeds_p1 = cp.tile([128, H], i32)
            nc.vector.tensor_scalar_add(seeds_p1[:], seeds_sb[:], 1)
            c_f = cp.tile([128, H], f32)
            nc.vector.tensor_copy(out=c_f[:], in_=seeds_p1[:])

            # hash indices: big_idx[p, a*8 + j], j = h*2 + (n-2)
            big_idx = cp.tile([128, NT * 8], i32)
            bi_view = big_idx[:].rearrange("p (a j) -> p a j", j=8)
            for h in range(H):
                ch = c_f[:, h : h + 1]
                s0 = wp.tile([128, NT], f32, tag="s0")
                s1 = wp.tile([128, NT], f32, tag="s1")
                s2 = wp.tile([128, NT], f32, tag="s2")
                nc.vector.tensor_scalar_mul(s0[:], Ts[0][:], ch)
                nc.vector.tensor_scalar_mul(s1[:], Ts[1][:], ch)
                nc.vector.tensor_scalar_mul(s2[:], Ts[2][:], ch)
                w2 = wp.tile([128, NT], f32, tag="w2")
                nc.vector.tensor_add(w2[:], s0[:], s1[:])
                w3 = wp.tile([128, NT], f32, tag="w3")
                nc.vector.tensor_add(w3[:], w2[:], s2[:])
                for bn, w in ((0, w2), (1, w3)):
                    j = h * 2 + bn
                    wi = wp.tile([128, NT], i32, tag="wi")
                    nc.vector.tensor_copy(out=wi[:], in_=w[:])
                    nc.vector.tensor_scalar(
                        out=bi_view[:, :, j],
                        in0=wi[:],
                        scalar1=HR - 1,
                        scalar2=None,
                        op0=OP.bitwise_and,
                    )

            # W_hid^T / H as (64 e, 512 d)
            wh_stg = cp.tile([128, 4 * E], f32)
            whv = w_hid[:].rearrange("(k p) e -> k p e", p=128)
            for k in range(4):
                nc.sync.dma_start(
                    out=wh_stg[:, k * E : (k + 1) * E], in_=whv[k]
                )
            whT = cp.tile([64, D], f32)
            for k in range(4):
                ps = ppA.tile([64, 128], f32, tag="tp")
                nc.tensor.transpose(
                    out=ps[:],
                    in_=wh_stg[:, k * E : (k + 1) * E],
                    identity=ident[:],
                )
                nc.vector.tensor_scalar_mul(
                    whT[:, k * 128 : (k + 1) * 128], ps[:], 1.0 / H
                )

            # W_g1^T as 4 k-tiles (128 d, 256 h2), stored (128, 4*256)
            wg1_stg = cp.tile([128, 2 * D], f32)
            wg1v = w_g1[:].rearrange("(m p) d -> m p d", p=128)
            for m in range(2):
                nc.sync.dma_start(
                    out=wg1_stg[:, m * D : (m + 1) * D], in_=wg1v[m]
                )
            wg1T = cp.tile([128, 4 * DH], f32)
            for k in range(4):
                for m in range(2):
                    ps = ppA.tile([128, 128], f32, tag="tp")
                    nc.tensor.transpose(
                        out=ps[:],
                        in_=wg1_stg[:, m * D + k * 128 : m * D + (k + 1) * 128],
                        identity=ident[:],
                    )
                    nc.vector.tensor_copy(
                        out=wg1T[:, k * DH + m * 128 : k * DH + (m + 1) * 128],
                        in_=ps[:],
                    )

            # W_g2^T and b_g1^T as (128, 2) column pairs
            wg2_stg = cp.tile([1, DH], f32)
            nc.sync.dma_start(out=wg2_stg[:], in_=w_g2[:])
            bg1_stg = cp.tile([1, DH], f32)
            nc.sync.dma_start(out=bg1_stg[:], in_=b_g1[:])
            wg2T = cp.tile([128, 2], f32)
            bg1T = cp.tile([128, 2], f32)
            for m in range(2):
                ps = ppA.tile([128, 1], f32, tag="tp")
                nc.tensor.transpose(
                    out=ps[:],
                    in_=wg2_stg[0:1, m * 128 : (m + 1) * 128],
                    identity=ident[0:1, 0:1],
                )
                nc.vector.tensor_copy(out=wg2T[:, m : m + 1], in_=ps[:])
                ps2 = ppA.tile([128, 1], f32, tag="tp")
                nc.tensor.transpose(
                    out=ps2[:],
                    in_=bg1_stg[0:1, m * 128 : (m + 1) * 128],
                    identity=ident[0:1, 0:1],
                )
                nc.vector.tensor_copy(out=bg1T[:, m : m + 1], in_=ps2[:])

            # b_hid as a row (added via K=1 matmul); b_g2 broadcast
            bhid_row = cp.tile([1, D], f32)
            nc.sync.dma_start(out=bhid_row[:], in_=b_hid[:])
            ones_row = cp.tile([1, 128], f32)
            nc.vector.memset(ones_row[:], 1.0)
            bg2_bc = cp.tile([128, 1], f32)
            nc.sync.dma_start(
                out=bg2_bc[:], in_=b_g2[:].to_broadcast((128, 1))
            )

            # masks for the final t-tile (invalid n-gram windows)
            mask2 = cp.tile([128, 1], f32)
            nc.vector.tensor_scalar(
                out=mask2[:], in0=ident[:, 127:128], scalar1=-1.0,
                scalar2=1.0, op0=OP.mult, op1=OP.add,
            )
            m3tmp = cp.tile([128, 1], f32)
            nc.vector.tensor_add(
                m3tmp[:], ident[:, 126:127], ident[:, 127:128]
            )
            mask3 = cp.tile([128, 1], f32)
            nc.vector.tensor_scalar(
                out=mask3[:], in0=m3tmp[:], scalar1=-1.0,
                scalar2=1.0, op0=OP.mult, op1=OP.add,
            )

            hidv = hid[:].rearrange("(a p) d -> a p d", p=128)
            outv = out[:].rearrange("(a p) d -> a p d", p=128)

            pair_state = {}

            def emit_tail(st):
                ap_j, ps_zt, mp_sbs, hid_sbs = st
                zg = wp.tile([128, 2 * DH], f32, tag="zg", name="zg")
                for m in range(2):
                    nc.scalar.activation(
                        out=zg[:, m * 2 * 128 : (m + 1) * 2 * 128],
                        in_=ps_zt[:, m * 256 : (m + 1) * 256],
                        func=AF.Gelu,
                        bias=bg1T[:, m : m + 1],
                    )
                ps_s = ppS.tile([128, 2], f32, tag="s", name="ps_s")
                for aoff in range(2):
                    for m in range(2):
                        nc.tensor.matmul(
                            ps_s[:, aoff : aoff + 1],
                            lhsT=zg[
                                :,
                                m * 2 * 128
                                + aoff * 128 : m * 2 * 128
                                + (aoff + 1) * 128,
                            ],
                            rhs=wg2T[:, m : m + 1],
                            start=(m == 0),
                            stop=(m == 1),
                        )
                gate = wp.tile([128, 2], f32, tag="gate", name="gate")
                nc.scalar.activation(
                    out=gate[:], in_=ps_s[:], func=AF.Sigmoid, bias=bg2_bc[:]
                )
                for aoff in range(2):
                    a = 2 * ap_j + aoff
                    o = wp.tile([128, D], f32, tag="o", name="o")
                    nc.vector.scalar_tensor_tensor(
                        out=o[:],
                        in0=mp_sbs[aoff][:],
                        scalar=gate[:, aoff : aoff + 1],
                        in1=hid_sbs[aoff][:],
                        op0=OP.mult,
                        op1=OP.add,
                    )
                    nc.sync.dma_start(out=outv[a], in_=o[:])

            def stage1(a):
                p = a // 2
                st = pair_state.setdefault(
                    p, {"mp": [None, None], "hid": [None, None],
                        "g": [None, None]}
                )
                gbuf = gp.tile([128, 8 * E], f32, tag="gbuf", name="gbuf")
                for j in range(8):
                    h = j // 2
                    nc.gpsimd.indirect_dma_start(
                        out=gbuf[:, j * E : (j + 1) * E],
                        out_offset=None,
                        in_=emb[:],
                        in_offset=IndirectOffsetOnAxis(
                            ap=big_idx[:, a * 8 + j : a * 8 + j + 1], axis=0
                        ),
                        element_offset=h * HR * E,
                    )
                if a == NT - 1:
                    for j in range(8):
                        msk = mask2 if j % 2 == 0 else mask3
                        nc.vector.tensor_scalar_mul(
                            gbuf[:, j * E : (j + 1) * E],
                            gbuf[:, j * E : (j + 1) * E],
                            msk[:],
                        )
                seqs = wp.tile([128, E], f32, tag="seqs", name="seqs")
                nc.vector.tensor_reduce(
                    out=seqs[:],
                    in_=gbuf[:].rearrange("p (j e) -> p e j", e=E),
                    axis=mybir.AxisListType.X,
                    op=OP.add,
                )
                ps_sqT = ppA.tile([64, 128], f32, tag="tp", name="ps_sqT")
                nc.tensor.transpose(
                    out=ps_sqT[:], in_=seqs[:], identity=ident[:]
                )
                sqT = wp.tile([64, 128], f32, tag="sqTs", name="sqT")
                nc.vector.tensor_copy(out=sqT[:], in_=ps_sqT[:])
                ps_mp = ppMP.tile([128, D], f32, tag="mp", name="ps_mp")
                nc.tensor.matmul(
                    ps_mp[:], lhsT=sqT[:], rhs=whT[:], start=True, stop=False
                )
                nc.tensor.matmul(
                    ps_mp[:], lhsT=ones_row[:], rhs=bhid_row[:],
                    start=False, stop=True,
                )
                mp_sb = hp.tile([128, D], f32, tag="mp_s", name="mp_sb")
                nc.vector.tensor_copy(out=mp_sb[:], in_=ps_mp[:])
                st["mp"][a % 2] = mp_sb
                hid_t = hp.tile([128, D], f32, tag="hid", name="hid_t")
                nc.sync.dma_start(out=hid_t[:], in_=hidv[a])
                st["hid"][a % 2] = hid_t
                g = hp.tile([128, D], f32, tag="g", name="g")
                nc.vector.tensor_add(g[:], hid_t[:], mp_sb[:])
                st["g"][a % 2] = g

            def stage2(a):
                p = a // 2
                st = pair_state[p]
                if "zall" not in st:
                    st["zall"] = ppZ.tile(
                        [128, 512], f32, tag="zm", name="ps_zall"
                    )
                ps_zall = st["zall"]
                g = st["g"][a % 2]
                gT = wp.tile([128, D], f32, tag="gT", name="gT")
                ps_g4 = ppG.tile([128, D], f32, tag="g4", name="ps_g4")
                for k in range(4):
                    nc.tensor.transpose(
                        out=ps_g4[:, k * 128 : (k + 1) * 128],
                        in_=g[:, k * 128 : (k + 1) * 128],
                        identity=ident[:],
                    )
                nc.vector.tensor_copy(out=gT[:], in_=ps_g4[:])
                aoff = a % 2
                for m in range(2):
                    for k in range(4):
                        nc.tensor.matmul(
                            ps_zall[
                                :,
                                m * 256 + aoff * 128 : m * 256 + (aoff + 1) * 128,
                            ],
                            lhsT=wg1T[
                                :, k * DH + m * 128 : k * DH + (m + 1) * 128
                            ],
                            rhs=gT[:, k * 128 : (k + 1) * 128],
                            start=(k == 0),
                            stop=(k == 3),
                        )

            def tail(p):
                st = pair_state.pop(p)
                emit_tail((p, st["zall"], st["mp"], st["hid"]))

            for a in range(NT + stag):
                if a < NT:
                    stage1(a)
                a2 = a - stag
                if 0 <= a2 < NT:
                    stage2(a2)
                    if a2 % 2 == 1:
                        pdone = a2 // 2
                        if pdone - tail_delay >= 0:
                            tail(pdone - tail_delay)
            for p in range(NT // 2 - tail_delay, NT // 2):
                tail(p)

    nc.compile()
    return nc


class _Runner:
    """PJRT runner (axon) for the prebuilt Bass module: emb + weights
    replicated to all cores, tok/hid sharded along the batch axis."""

    REPLICATED = {"emb", "w_hid", "b_hid", "w_g1", "b_g1", "w_g2", "b_g2",
                  "seeds", "ident"}

    def __init__(self, nc):
        import jax
        from jax.sharding import Mesh, NamedSharding, PartitionSpec
        from jax.experimental.shard_map import shard_map
        import concourse.mybir as mybir
        from concourse import bass2jax

        self.jax = jax
        self.NamedSharding = NamedSharding
        self.PartitionSpec = PartitionSpec
        bass2jax.install_neuronx_cc_hook()
        self.nc = nc
        partition_name = (
            nc.partition_id_tensor.name if nc.partition_id_tensor else None
        )
        in_names, out_names, out_avals, zero_outs = [], [], [], []
        for alloc in nc.m.functions[0].allocations:
            if not isinstance(alloc, mybir.MemoryLocationSet):
                continue
            name = alloc.memorylocations[0].name
            if alloc.kind == "ExternalInput":
                if name != partition_name:
                    in_names.append(name)
            elif alloc.kind == "ExternalOutput":
                out_names.append(name)
                shape = tuple(alloc.tensor_shape)
                dtype = mybir.dt.np(alloc.dtype)
                out_avals.append(jax.core.ShapedArray(shape, dtype))
                zero_outs.append(np.zeros(shape, dtype))
        self.in_names = in_names
        self.out_names = out_names
        self.out_avals = out_avals
        self.zero_outs = zero_outs
        n_params = len(in_names)
        n_outs = len(out_avals)
        all_names = list(in_names) + list(out_names)
        if partition_name is not None:
            all_names.append(partition_name)
        all_names = tuple(all_names)

        def _body(*args):
            operands = list(args)
            if partition_name is not None:
                operands.append(bass2jax.partition_id_tensor())
            outs = bass2jax._bass_exec_p.bind(
                *operands,
                out_avals=tuple(out_avals),
                in_names=all_names,
                out_names=tuple(out_names),
                lowering_input_output_aliases=(),
                sim_require_finite=True,
                sim_require_nnan=True,
                nc=nc,
            )
            return tuple(outs)

        devices = jax.devices()[:N_CORES]
        self.mesh = Mesh(np.asarray(devices), ("core",))
        in_specs = tuple(
            PartitionSpec() if name in self.REPLICATED
            else PartitionSpec("core")
            for name in in_names
        ) + (PartitionSpec("core"),) * n_outs
        out_specs = (PartitionSpec("core"),) * n_outs
        self.fn = jax.jit(
            shard_map(
                _body, mesh=self.mesh, in_specs=in_specs,
                out_specs=out_specs, check_rep=False,
            ),
            donate_argnums=tuple(range(n_params, n_params + n_outs)),
            keep_unused=True,
        )

    def _sharding(self, name=None):
        if name is not None and name in self.REPLICATED:
            return self.NamedSharding(self.mesh, self.PartitionSpec())
        return self.NamedSharding(self.mesh, self.PartitionSpec("core"))

    def put_inputs(self, per_core, replicated_map):
        arrs = []
        for name in self.in_names:
            if name in self.REPLICATED:
                a = replicated_map[name]
            else:
                a = np.concatenate([m[name] for m in per_core], axis=0)
            arrs.append(self.jax.device_put(a, self._sharding(name)))
        self.jax.block_until_ready(arrs)
        return arrs

    def put_zeros(self):
        zs = []
        for z in self.zero_outs:
            full = np.zeros((N_CORES * z.shape[0], *z.shape[1:]), z.dtype)
            zs.append(self.jax.device_put(full, self._sharding()))
        self.jax.block_until_ready(zs)
        return zs

    def run(self, dev_inputs):
        outs = self.fn(*dev_inputs, *self.put_zeros())
        self.jax.block_until_ready(outs)
        full = np.asarray(outs[0]).reshape(N_CORES, T, D)
        return full


def _get_runner():
    if "runner" not in _CACHE:
        nc = _build_nc()
        _CACHE["runner"] = _Runner(nc)
    return _CACHE["runner"]


def kernel(token_ids, hidden_state, embeddings, W_hid, b_hid, W_g1, b_g1,
           W_g2, b_g2, seeds, hash_range, max_n):
    token_ids = np.asarray(token_ids, np.int32)
    hidden_state = np.asarray(hidden_state, np.float32)
    embeddings = np.asarray(embeddings, np.float32)
    assert int(hash_range) == HR and int(max_n) == 3
    assert token_ids.shape == (B, T) and hidden_state.shape == (B, T, D)

    replicated = {
        "emb": embeddings.reshape(H * HR, E),
        "w_hid": np.asarray(W_hid, np.float32).reshape(D, E),
        "b_hid": np.asarray(b_hid, np.float32).reshape(1, D),
        "w_g1": np.asarray(W_g1, np.float32).reshape(DH, D),
        "b_g1": np.asarray(b_g1, np.float32).reshape(1, DH),
        "w_g2": np.asarray(W_g2, np.float32).reshape(1, DH),
        "b_g2": np.asarray(b_g2, np.float32).reshape(1, 1),
        "seeds": np.asarray(seeds, np.int32).reshape(1, H),
        "ident": np.eye(128, dtype=np.float32),
    }
    per_core = [
        {"tok": token_ids[c : c + 1], "hid": hidden_state[c]}
        for c in range(N_CORES)
    ]

    r = _get_runner()
    dev = r.put_inputs(per_core, replicated)
    return r.run(dev)
